# revision 20
# baseline (speedup 1.0000x reference)
"""Causal self-attention (B=2, S=2048, D=1024, H=16) on 8 TRN2 NeuronCores.

Sharding (Megatron-style, per the hint): 2 batches x 4 head-groups -> 8 cores.
Core c handles batch b = c // 4 and local heads [4*(c%4), 4*(c%4)+4).

Per-core device program (single NEFF, SPMD with per-core input shards):
  QT = Wq_g.T @ x_b.T          [256, 2048]  (head-dim on partitions)
  KT = Wk_g.T @ x_b.T          [256, 2048]
  V  = x_b @ Wv_g              [2048, 256]  (seq on partitions), augmented
                               with a ones column per head for the softmax
                               denominator.
  per q-block of 512 (both head pairs interleaved, ST one k-tile ahead of
  PV so the PE never stalls on the exp/mask chain):
    ST[k,q] = K_h Q_h^T        (transposed scores, k on partitions)
    E       = exp(ST/8) * causal_mask
    ctxT[hd+1, q] += V_aug_h[kblock].T @ E   (PSUM accumulate over k blocks;
                                              row hd holds the denominator)
    ctxn = ctxT[0:hd] * (1/den broadcast)    (partition-broadcast via a
                                              ones-vector PE matmul)
  V-projection tile blocks are emitted between q-blocks so the PE chews
  them while the ACT engine (exp) is the attention bottleneck.
  y_partial[q, :] = sum_pair cn2_pair.T @ Wo_pair   (128-deep contraction,
                                                     head pairs packed)
Host sums the 4 partial y's per batch (tensor-parallel reduction on host).
"""

import os
import sys

if "/opt/trn_rl_repo" not in sys.path:
    sys.path.insert(0, "/opt/trn_rl_repo")

from contextlib import ExitStack

import numpy as np

import concourse.bass as bass
import concourse.mybir as mybir
import concourse.tile as tile
from concourse import bacc
from concourse.bass_utils import run_bass_kernel_spmd

B, S, D, H, HD = 2, 2048, 1024, 16, 64
HPC = 4            # heads per core
CD = HPC * HD      # 256: per-core projection width
NCORES = 8
QB = 512           # q block size (one PSUM bank of fp32)
NDT = D // 128     # 8 contraction tiles for projections
NKT = S // 128     # 16 seq tiles
f32 = mybir.dt.float32
f32r = mybir.dt.float32r
bf16 = mybir.dt.bfloat16
EXP = mybir.ActivationFunctionType.Exp


def _build(tc, xT, wq, wk, wv, wo, msk, y):
    nc = tc.nc
    abl = os.environ.get("KERNEL_ABL", "")

    with ExitStack() as top:
        singles = top.enter_context(tc.tile_pool(name="singles", bufs=1))
        QT_sb = [singles.tile([128, S], bf16, name=f"qtsb{m}", tag=f"qtsb{m}") for m in range(2)]
        KT_sb = [singles.tile([128, S], bf16, name=f"ktsb{m}", tag=f"ktsb{m}") for m in range(2)]
        V4 = singles.tile([128, NKT, HPC, HD + 1], bf16, name="v4", tag="v4")
        masks = singles.tile([128, 4 * QB], bf16, name="masks", tag="masks")
        wo_sb = singles.tile([128, 2, D], bf16, name="wo_sb", tag="wo_sb")
        ones_sb = singles.tile([HD + 1, HD], f32r, name="ones_sb", tag="ones_sb")
        # ones columns of the augmented V (denominator accumulators)
        nc.vector.memset(V4[:, :, :, HD:HD + 1], 1.0)
        nc.vector.memset(ones_sb.bitcast(f32), 1.0)

        # weights + x stay resident for the whole kernel.  DMA order matters:
        # the first Q-proj matmul needs only wq + x tile 0, so issue those
        # first; masks/wo aren't needed until the first exp / out-proj.
        pw = top.enter_context(tc.tile_pool(name="projw", bufs=1))
        wsb = {}
        for nm in ("wq", "wk", "wv"):
            wsb[nm] = pw.tile([128, NDT, CD], bf16, name=f"{nm}sb", tag=f"{nm}sb")
        xsb = pw.tile([128, NDT, S], bf16, name="xsb", tag="xsb")
        xr = xT.rearrange("(kt p) s -> p kt s", p=128)
        # x lands in consumption order: all 8 k-tiles of seq-chunk sc feed
        # proj_sc(sc) (and V tiles 4sc..4sc+3), so sc0 is fully fed ~4us in
        nc.sync.dma_start(wsb["wq"], wq.rearrange("(kt p) c -> p kt c", p=128))
        for kt in range(NDT):
            nc.sync.dma_start(xsb[:, kt, 0:QB], xr[:, kt, 0:QB])
        nc.sync.dma_start(wsb["wk"], wk.rearrange("(kt p) c -> p kt c", p=128))
        for kt in range(NDT):
            nc.sync.dma_start(xsb[:, kt, QB:2 * QB], xr[:, kt, QB:2 * QB])
        nc.sync.dma_start(wsb["wv"], wv.rearrange("(kt p) c -> p kt c", p=128))
        for sc in range(2, S // QB):
            for kt in range(NDT):
                nc.sync.dma_start(xsb[:, kt, sc * QB:(sc + 1) * QB],
                                  xr[:, kt, sc * QB:(sc + 1) * QB])
        nc.sync.dma_start(masks, msk)
        nc.sync.dma_start(wo_sb, wo.rearrange("pr p c -> p pr c"))

        # ---------------- attention + interleaved projections ----------------
        if abl == "noproj":
            return
        with ExitStack() as att:
            stp = att.enter_context(tc.tile_pool(name="stp", bufs=2, space="PSUM"))
            accp = att.enter_context(tc.tile_pool(name="accp", bufs=4, space="PSUM"))
            ep = att.enter_context(tc.tile_pool(name="ep", bufs=6))
            normp = att.enter_context(tc.tile_pool(name="normp", bufs=4))
            cnp = att.enter_context(tc.tile_pool(name="cnp", bufs=4))
            ysbp = att.enter_context(tc.tile_pool(name="ysbp", bufs=4))
            warmp = att.enter_context(tc.tile_pool(name="warmp", bufs=1))

            # tiny warmup exp so the implicit activation-table load runs at
            # t~0 instead of delaying the first real exp
            warm = warmp.tile([1, 8], f32, name="warm", tag="warm")
            warme = warmp.tile([1, 8], bf16, name="warme", tag="warme")
            nc.vector.memset(warm, 0.0)
            nc.scalar.activation(warme, warm, EXP, scale=0.125)

            def proj_sc(sc):
                # Q/K projection for seq columns [sc*QB, (sc+1)*QB): 4 PSUM
                # accumulators (Q/K x m-half), copies to SBUF on DVE
                accs = {}
                for (nm, T_sb) in (("wq", QT_sb), ("wk", KT_sb)):
                    for m in range(2):
                        accs[nm, m] = accp.tile([128, QB], f32,
                                                name=f"pj{nm}{m}", tag="acc")
                for kt in range(NDT):
                    for (nm, m) in accs:
                        nc.tensor.matmul(
                            accs[nm, m],
                            wsb[nm][:, kt, m * 128:(m + 1) * 128],
                            xsb[:, kt, sc * QB:(sc + 1) * QB],
                            start=(kt == 0), stop=(kt == NDT - 1),
                        )
                for ((nm, m), acc) in accs.items():
                    T_sb = QT_sb if nm == "wq" else KT_sb
                    nc.vector.tensor_copy(T_sb[m][:, sc * QB:(sc + 1) * QB], acc)

            def v_tile(st):
                # V: out[s_tile, 4*64] = x @ Wv, PSUM acc rotates through stp
                psv = stp.tile([128, CD], f32, name="psv", tag="st")
                for kt in range(NDT):
                    nc.tensor.matmul(
                        psv,
                        xsb[:, kt, st * 128:(st + 1) * 128],
                        wsb["wv"][:, kt, :],
                        start=(kt == 0), stop=(kt == NDT - 1),
                    )
                nc.vector.tensor_copy(
                    V4[:, st, :, 0:HD],
                    psv.rearrange("p (h d) -> p h d", h=HPC),
                )

            if abl == "projonly":
                proj_sc(0)
                for st in range(4):
                    v_tile(st)
                proj_sc(1)
                return

            def emit_st(qb, kt, st_t):
                # causal band: columns q < rel*128 of this k-tile are fully
                # masked; skip them in ST/exp/PV (the stale-but-bounded PSUM
                # region is never read because the PV rhs is trimmed)
                rel = kt - 4 * qb
                lo = rel * 128 if rel > 0 else 0
                for pair in range(2):
                    QTp, KTp = QT_sb[pair], KT_sb[pair]
                    stT = stp.tile([128, 2 * QB], f32, name="stT", tag="st")
                    # head A on PE rows 0-63, head B on rows 64-127
                    nc.tensor.matmul(
                        stT[:, lo:QB],
                        KTp[0:HD, kt * 128:(kt + 1) * 128],
                        QTp[0:HD, qb * QB + lo:(qb + 1) * QB],
                        start=True, stop=True,
                    )
                    nc.tensor.matmul(
                        stT[:, QB + lo:2 * QB],
                        KTp[HD:128, kt * 128:(kt + 1) * 128],
                        QTp[HD:128, qb * QB + lo:(qb + 1) * QB],
                        start=True, stop=True,
                    )
                    st_t[pair] = stT

            def emit_exp(qb, kt, st_t, eT):
                rel = kt - 4 * qb
                lo = rel * 128 if rel > 0 else 0
                for pair in range(2):
                    stT = st_t[pair]
                    e = ep.tile([128, 2 * QB], bf16, name="eT", tag="e")
                    if lo == 0:
                        nc.scalar.activation(e, stT, EXP, scale=0.125)
                    else:
                        nc.scalar.activation(e[:, lo:QB], stT[:, lo:QB],
                                             EXP, scale=0.125)
                        nc.scalar.activation(e[:, QB + lo:2 * QB],
                                             stT[:, QB + lo:2 * QB],
                                             EXP, scale=0.125)
                    if rel >= 0:
                        # only the first 128 band columns are partially
                        # masked; beyond them every k-row is causal-valid
                        msl = masks[:, rel * QB + lo:rel * QB + lo + 128]
                        nc.vector.tensor_mul(e[:, lo:lo + 128],
                                             e[:, lo:lo + 128], msl)
                        nc.vector.tensor_mul(e[:, QB + lo:QB + lo + 128],
                                             e[:, QB + lo:QB + lo + 128], msl)
                    eT[pair] = e

            def emit_pv(qb, kt, nkt, eT, ctx):
                rel = kt - 4 * qb
                lo = rel * 128 if rel > 0 else 0
                for pair in range(2):
                    e = eT[pair]
                    nc.tensor.matmul(
                        ctx[2 * pair][:, lo:QB], V4[:, kt, 2 * pair, :],
                        e[:, lo:QB],
                        start=(kt == 0), stop=(kt == nkt - 1), skip_group_check=True,
                    )
                    nc.tensor.matmul(
                        ctx[2 * pair + 1][:, lo:QB], V4[:, kt, 2 * pair + 1, :],
                        e[:, QB + lo:2 * QB],
                        start=(kt == 0), stop=(kt == nkt - 1), skip_group_check=True,
                    )

            def normalize(qb, ctx):
                # cn2[pair] packs both heads on 128 partitions for the
                # 128-deep out-projection contraction
                cn2 = [cnp.tile([128, QB], bf16, name=f"cn{p}", tag="cn")
                       for p in range(2)]
                for pair in range(2):
                    for h01 in range(2):
                        ctx_t = ctx[2 * pair + h01]
                        recip = normp.tile([HD + 1, QB], f32r, name="recip", tag="recip")
                        with nc.allow_low_precision(reason="f32r 1/den for PE broadcast"):
                            nc.vector.reciprocal(recip[HD:HD + 1, :], ctx_t[HD:HD + 1, :])
                        bcps = stp.tile([HD, QB], f32, name="bcps", tag="st")
                        nc.tensor.matmul(
                            bcps, ones_sb[HD:HD + 1, :], recip[HD:HD + 1, :],
                            start=True, stop=True, skip_group_check=True,
                        )
                        # stage ctx out of PSUM on ACT (idle at boundaries) so
                        # the DVE mul has a single PSUM operand
                        ctxc = normp.tile([HD, QB], bf16, name="ctxc", tag="ctxc")
                        nc.scalar.copy(ctxc, ctx_t[0:HD, :])
                        nc.vector.tensor_mul(
                            cn2[pair][h01 * HD:(h01 + 1) * HD, :],
                            ctxc, bcps)
                return cn2

            def out_proj(qb, cn2):
                # output projection for this q block (pairs packed, K=128)
                for qt in range(QB // 128):
                    yps = [accp.tile([128, 512], f32, name=f"yp{nh}", tag="acc")
                           for nh in range(2)]
                    for nh in range(2):
                        for pair in range(2):
                            nc.tensor.matmul(
                                yps[nh],
                                cn2[pair][:, qt * 128:(qt + 1) * 128],
                                wo_sb[:, pair, nh * 512:(nh + 1) * 512],
                                start=(pair == 0), stop=(pair == 1),
                                skip_group_check=True,
                            )
                    for nh in range(2):
                        ysb = ysbp.tile([128, 512], f32, name="ysb", tag="ysb")
                        nc.vector.tensor_copy(ysb, yps[nh])
                        nc.sync.dma_start(
                            y[qb * QB + qt * 128: qb * QB + (qt + 1) * 128,
                              nh * 512:(nh + 1) * 512],
                            ysb,
                        )

            # flat (qb, kt) sequence with cross-boundary ST lookahead: the
            # next q-block's first score matmul is emitted before this block's
            # boundary work so the ACT engine keeps chewing exps while the PE
            # runs normalize/proj/out-proj
            seq = [(qb, kt) for qb in range(S // QB) for kt in range(4 * (qb + 1))]
            ctxs, st_t, eT = {}, {}, {}
            # prefix: sc0 proj feeds the first score matmul as early as
            # possible so the ACT exp chain starts ~10us in, not ~29us;
            # sc1 proj then runs on the PE underneath the first exps
            proj_sc(0)
            for st in range(4):
                v_tile(st)
            emit_st(0, 0, st_t)
            proj_sc(1)
            for i, (qb, kt) in enumerate(seq):
                nkt = 4 * (qb + 1)
                if kt == 0:
                    ctxs[qb] = [accp.tile([HD + 1, QB], f32, name=f"ctx{h}", tag="acc")
                                for h in range(HPC)]
                emit_exp(qb, kt, st_t, eT)
                if i + 1 < len(seq):
                    emit_st(seq[i + 1][0], seq[i + 1][1], st_t)
                emit_pv(qb, kt, nkt, eT, ctxs[qb])
                # PE filler: project the next q-block's new V tiles, spread
                # through the loop (ACT exp is the per-iteration laggard)
                if qb + 1 < S // QB and kt in ((j + 1) * nkt // 5 for j in range(4)):
                    j = [(jj + 1) * nkt // 5 for jj in range(4)].index(kt)
                    v_tile(4 * (qb + 1) + j)
                if kt == nkt - 1:
                    cn2 = normalize(qb, ctxs.pop(qb))
                    if qb + 2 < S // QB:
                        proj_sc(qb + 2)
                    if abl != "noout":
                        out_proj(qb, cn2)


def build_bass(reps=1):
    nc = bacc.Bacc("TRN2", target_bir_lowering=False, debug=False,
                   num_devices=NCORES)
    xT = nc.dram_tensor("xt", [D, S], bf16, kind="ExternalInput").ap()
    wq = nc.dram_tensor("wq", [D, CD], bf16, kind="ExternalInput").ap()
    wk = nc.dram_tensor("wk", [D, CD], bf16, kind="ExternalInput").ap()
    wv = nc.dram_tensor("wv", [D, CD], bf16, kind="ExternalInput").ap()
    wo = nc.dram_tensor("wo", [2, 128, D], bf16, kind="ExternalInput").ap()
    msk = nc.dram_tensor("msk", [128, 4 * QB], bf16, kind="ExternalInput").ap()
    y = nc.dram_tensor("y", [S, D], f32, kind="ExternalOutput").ap()
    with tile.TileContext(nc) as tc:
        for _ in range(reps):
            _build(tc, xT, wq, wk, wv, wo, msk, y)
    nc.compile()
    return nc


import ml_dtypes

BF = ml_dtypes.bfloat16


def _causal_masks():
    # masks[k, rel*QB + q] = 1.0 iff rel*128 + k <= q   (rel = k-tile index
    # inside the q block)
    k = np.arange(128)[:, None]
    q = np.arange(QB)[None, :]
    cols = [(rel * 128 + k <= q).astype(BF) for rel in range(4)]
    return np.concatenate(cols, axis=1)


def make_in_maps(x, Wq, Wk, Wv, Wo):
    msk = _causal_masks()
    in_maps = []
    for c in range(NCORES):
        b, g = divmod(c, 4)
        cs = slice(g * CD, (g + 1) * CD)
        in_maps.append({
            "xt": np.ascontiguousarray(x[b].T).astype(BF),
            "wq": np.ascontiguousarray(Wq[:, cs]).astype(BF),
            "wk": np.ascontiguousarray(Wk[:, cs]).astype(BF),
            "wv": np.ascontiguousarray(Wv[:, cs]).astype(BF),
            "wo": np.ascontiguousarray(Wo[cs, :]).reshape(2, 128, D).astype(BF),
            "msk": msk,
        })
    return in_maps


_NC_CACHE = None


def get_nc():
    global _NC_CACHE
    if _NC_CACHE is None:
        _NC_CACHE = build_bass()
    return _NC_CACHE


def kernel(x, Wq, Wk, Wv, Wo, trace=False, **trace_kwargs):
    x = np.asarray(x, dtype=np.float32)
    in_maps = make_in_maps(x, np.asarray(Wq, np.float32), np.asarray(Wk, np.float32),
                           np.asarray(Wv, np.float32), np.asarray(Wo, np.float32))
    res = run_bass_kernel_spmd(get_nc(), in_maps, core_ids=list(range(NCORES)),
                               trace=trace, **trace_kwargs)
    parts = [r["y"] for r in res.results]
    out = np.empty((B, S, D), dtype=np.float32)
    for b in range(B):
        out[b] = parts[4 * b] + parts[4 * b + 1] + parts[4 * b + 2] + parts[4 * b + 3]
    kernel.last_results = res
    return out
